# revision 7
# baseline (speedup 1.0000x reference)
"""Trainium2 Bass kernel for nn_BSplineActivation.

y(x) = sum_j B_j(x) w_j for a degree-3 B-spline on a uniform knot grid
(1024 knots on [-pi, pi], fp32). Per point only 4 basis functions are
non-zero, so y restricted to knot interval i is a cubic polynomial.

Strategy:
  * Host (weights-only preprocessing): build a [1024, 64] table whose
    row i holds the exact cubic coefficients (c0..c3, zero padded to a
    256B DMA row) of y in the normalized local coordinate s = z - i,
    z = x*inv_h + cb. Built in float64 from the float32 knot values.
  * Device (all per-point work): z = x*inv_h + C, clamp, floor -> i
    (f32->i32 conversion truncates, which is floor for z >= 0);
    gather row i for all 32768 points with a single SWDGE dma_gather
    (int16 indices in the 16-wrapped layout, output lands in natural
    [128, 256, 64] layout); s = z - i; Horner; mask outside
    [knot0, knot_last).
  * Data parallel over 8 NeuronCores: x is split into 8 shards of 32768
    points; the table is replicated.

Point layout per core: point (p, f) of the [128, 256] shard has gather
list position i = p + 128*f, so its index must be stored at
[i % 16, i // 16] = [p % 16, p//16 + 8*f] (16-wrapped layout) and its
gathered row lands at [i % 128, i // 128] = [p, f] (natural layout).
The host uploads x twice: natural [128, 256] and wrapped+replicated
[128, 2048] (each 16-partition group holds the same wrapped block).
"""
import sys

sys.path.insert(0, "/opt/trn_rl_repo")

import numpy as np

import concourse.bacc as bacc
import concourse.mybir as mybir
import concourse.tile as tile
from concourse.bass_utils import run_bass_kernel_spmd

P, F = 128, 256          # per-core layout: 128 partitions x 256 points
NCORES = 8
NPTS = NCORES * P * F    # 262144
NPC = P * F              # 32768 points per core
FW = NPC // 16           # 2048 wrapped-index columns
NUM_KNOTS = 1024
DEGREE = 3
NI = NUM_KNOTS - 1       # 1023 intervals
ROW = 64                 # table row: 64 f32 = 256B (dma_gather minimum)

f32 = mybir.dt.float32
i32 = mybir.dt.int32
i16 = mybir.dt.int16
AL = mybir.AluOpType
AF = mybir.ActivationFunctionType

_KNOTS32 = np.linspace(-np.pi, np.pi, NUM_KNOTS).astype(np.float32)
_T0 = float(_KNOTS32[0])
_TLAST = float(_KNOTS32[-1])
# z = x * INV_H + CB maps x to the (approximate) interval coordinate.
_H64 = (float(_KNOTS32[-1]) - float(_KNOTS32[0])) / float(NI)
_INV_H = float(np.float32(1.0 / _H64))
_CB = float(np.float32(-float(_KNOTS32[0]) / _H64))


def _bspline_basis_f64(x, knots, degree):
    """Reference Cox-de Boor recursion in float64 (on fp32 knot values)."""
    t = knots.astype(np.float64)
    n = t.shape[0] - 1
    xe = x[:, None]
    B = ((t[:-1] <= xe) & (xe < t[1:])).astype(np.float64)
    for k in range(1, degree + 1):
        d1 = t[k:n] - t[: n - k]
        d2 = t[k + 1 : n + 1] - t[1 : n - k + 1]
        w1 = np.where(d1 > 0, (xe - t[: n - k]) / np.where(d1 > 0, d1, 1.0), 0.0)
        w2 = np.where(d2 > 0, (t[k + 1 : n + 1] - xe) / np.where(d2 > 0, d2, 1.0), 0.0)
        B = w1 * B[:, : n - k] + w2 * B[:, 1 : n - k + 1]
    return B


def _build_table(weights: np.ndarray) -> np.ndarray:
    """[1024, 64] fp32: row i holds cubic coeffs c0..c3 in s = z - i."""
    w64 = weights.astype(np.float64)
    # 4 sample offsets inside each interval (fractions of the local width)
    fr = np.array([0.0625, 0.3125, 0.6875, 0.9375])
    t64 = _KNOTS32.astype(np.float64)
    lo = t64[:-1]
    wid = t64[1:] - t64[:-1]
    xs = lo[:, None] + wid[:, None] * fr[None, :]          # [1023, 4]
    ys = _bspline_basis_f64(xs.ravel(), _KNOTS32, DEGREE) @ w64
    ys = ys.reshape(NI, 4)
    # The device's s at sample x: s = (x - t0)/h - i.
    zs = (xs - float(_KNOTS32[0])) / _H64
    ss = zs - np.arange(NI)[:, None]                       # [1023, 4] ~ fr
    V = np.stack([ss**k for k in range(4)], axis=-1)       # [1023, 4, 4]
    a = np.linalg.solve(V, ys[:, :, None])[:, :, 0]        # [1023, 4]
    tab = np.zeros((NUM_KNOTS, ROW), dtype=np.float32)
    tab[:NI, :4] = a.astype(np.float32)
    return tab


_NC_CACHE = {}


def _build_nc(reps: int = 1):
    """reps > 1 repeats the full compute body (for device-time estimation)."""
    nc = bacc.Bacc("TRN2", target_bir_lowering=False, debug=False, num_devices=NCORES)
    x_d = nc.dram_tensor("x", [P, F], f32, kind="ExternalInput")
    x16_d = nc.dram_tensor("x16", [P, FW], f32, kind="ExternalInput")
    tab_d = nc.dram_tensor("tab", [NUM_KNOTS, ROW], f32, kind="ExternalInput")
    y_d = nc.dram_tensor("y", [P, F], f32, kind="ExternalOutput")
    with tile.TileContext(nc) as tc:
        with tc.tile_pool(name="sbuf", bufs=1) as pool:
            xt = pool.tile([P, F], f32)
            x16t = pool.tile([P, FW], f32)
            nc.sync.dma_start(xt[:], x_d.ap()[:])
            nc.sync.dma_start(x16t[:], x16_d.ap()[:])
            # tiles allocated once, reused by every rep
            z16 = pool.tile([P, FW], f32)
            idx16i = pool.tile([P, FW], i32)
            idx16 = pool.tile([P, FW], i16)
            zt = pool.tile([P, F], f32)
            idxi = pool.tile([P, F], i32)
            idxf = pool.tile([P, F], f32)
            st = pool.tile([P, F], f32)
            gath = pool.tile([P, F * ROW], f32)
            acc = pool.tile([P, F], f32)
            m1 = pool.tile([P, F], f32)
            yt = pool.tile([P, F], f32)
            for _rep in range(reps):
                # --- wrapped-index path (feeds the gather) ---
                # z = x*inv_h + cb; clamp; floor = round(z - 0.5) (the HW
                # f32->int convert rounds to nearest)
                nc.vector.tensor_scalar(out=z16[:], in0=x16t[:], scalar1=_INV_H,
                                        scalar2=_CB, op0=AL.mult, op1=AL.add)
                nc.vector.tensor_scalar(out=z16[:], in0=z16[:], scalar1=0.0,
                                        scalar2=1022.9995, op0=AL.max, op1=AL.min)
                nc.vector.tensor_scalar(out=idx16i[:], in0=z16[:], scalar1=0.5,
                                        scalar2=None, op0=AL.subtract)
                nc.vector.tensor_copy(out=idx16[:], in_=idx16i[:])
                # --- gather: 32768 rows of 256B from the DRAM table ---
                # Chunked: the SWDGE gather ucode handles at most 1024
                # indices per call (measured on HW; >1024 faults), so
                # 32 calls of 8 output columns each.
                gv3 = gath[:].rearrange("p (f e) -> p f e", e=ROW)
                for c0 in range(0, F, 8):
                    cols = min(8, F - c0)
                    nidx = cols * P
                    nc.gpsimd.dma_gather(
                        out_ap=gv3[:, c0 : c0 + cols, :],
                        in_ap=tab_d.ap()[:],
                        idxs_ap=idx16[:, c0 * 8 : (c0 + cols) * 8],
                        num_idxs=nidx,
                        num_idxs_reg=nidx,
                        elem_size=ROW,
                    )
                # --- natural-layout path (s and masks; overlaps the gather) ---
                nc.vector.tensor_scalar(out=zt[:], in0=xt[:], scalar1=_INV_H,
                                        scalar2=_CB, op0=AL.mult, op1=AL.add)
                nc.vector.tensor_scalar(out=zt[:], in0=zt[:], scalar1=0.0,
                                        scalar2=1022.9995, op0=AL.max, op1=AL.min)
                nc.vector.tensor_scalar(out=idxi[:], in0=zt[:], scalar1=0.5,
                                        scalar2=None, op0=AL.subtract)
                nc.vector.tensor_copy(out=idxf[:], in_=idxi[:])
                nc.vector.tensor_tensor(out=st[:], in0=zt[:], in1=idxf[:], op=AL.subtract)
                # --- Horner on the gathered coefficients ---
                gv = gath[:].rearrange("p (f e) -> p f e", e=ROW)
                nc.vector.tensor_tensor(out=acc[:], in0=gv[:, :, 3], in1=st[:], op=AL.mult)
                nc.vector.tensor_tensor(out=acc[:], in0=acc[:], in1=gv[:, :, 2], op=AL.add)
                nc.vector.tensor_tensor(out=acc[:], in0=acc[:], in1=st[:], op=AL.mult)
                nc.vector.tensor_tensor(out=acc[:], in0=acc[:], in1=gv[:, :, 1], op=AL.add)
                nc.vector.tensor_tensor(out=acc[:], in0=acc[:], in1=st[:], op=AL.mult)
                nc.vector.tensor_tensor(out=acc[:], in0=acc[:], in1=gv[:, :, 0], op=AL.add)
                # mask to zero outside [T0, TLAST)
                nc.vector.scalar_tensor_tensor(out=m1[:], in0=xt[:], scalar=_TLAST,
                                               in1=acc[:], op0=AL.is_lt, op1=AL.mult)
                nc.vector.scalar_tensor_tensor(out=yt[:], in0=xt[:], scalar=_T0,
                                               in1=m1[:], op0=AL.is_ge, op1=AL.mult)
            nc.sync.dma_start(y_d.ap()[:], yt[:])
    nc.compile()
    return nc


def _wrap16(xs: np.ndarray) -> np.ndarray:
    """[128, 256] natural -> [128, 2048] wrapped (replicated per 16-part group).

    Point (p, f) -> wrapped [p % 16, p//16 + 8*f]; the [16, 2048] block is
    then replicated to all 8 groups of 16 partitions.
    """
    v = xs.reshape(8, 16, F)                      # [pg, r, f]
    w = np.transpose(v, (1, 2, 0)).reshape(16, F * 8)  # [r, (f pg)]
    return np.ascontiguousarray(np.tile(w, (8, 1)))


def _in_maps(x, weights):
    tab = _build_table(np.asarray(weights))
    xs = np.ascontiguousarray(np.asarray(x, dtype=np.float32).reshape(NCORES, P, F))
    return [{"x": xs[c], "x16": _wrap16(xs[c]), "tab": tab} for c in range(NCORES)]


def kernel(x: np.ndarray, weights: np.ndarray) -> np.ndarray:
    if "nc" not in _NC_CACHE:
        _NC_CACHE["nc"] = _build_nc()
    nc = _NC_CACHE["nc"]
    res = run_bass_kernel_spmd(nc, _in_maps(x, weights), core_ids=list(range(NCORES)))
    y = np.stack([res.results[c]["y"] for c in range(NCORES)], axis=0)
    return y.reshape(NPTS, 1).astype(np.float32)


def estimate_hw_ns(x=None, weights=None, reps_hi: int = 3, timing_reps: int = 12) -> int:
    """Device time per kernel body: wall-clock delta between reps=1 and
    reps=reps_hi builds (amplification cancels host/launch overhead)."""
    import time as _time

    if x is None:
        rng = np.random.default_rng(0)
        x = rng.standard_normal((NPTS, 1)).astype(np.float32)
        weights = rng.standard_normal((1020,)).astype(np.float32)
    im = _in_maps(x, weights)
    walls = {}
    for reps in (1, reps_hi):
        nc = _NC_CACHE.get(("nc", reps))
        if nc is None:
            nc = _build_nc(reps) if reps > 1 else _NC_CACHE.get("nc") or _build_nc()
            _NC_CACHE[("nc", reps)] = nc
        run_bass_kernel_spmd(nc, im, core_ids=list(range(NCORES)))
        ts = []
        for _ in range(timing_reps):
            t0 = _time.perf_counter()
            run_bass_kernel_spmd(nc, im, core_ids=list(range(NCORES)))
            ts.append(_time.perf_counter() - t0)
        walls[reps] = min(ts)
    return int((walls[reps_hi] - walls[1]) / (reps_hi - 1) * 1e9)
